# revision 1
# baseline (speedup 1.0000x reference)
"""2-layer GCN on 8 Trainium2 NeuronCores.

Strategy (edge-cut node sharding, per sharding_hint):
- 8 cores, core c owns dst nodes [c*12500, (c+1)*12500).
- Per layer, each core builds its shard of the message table xs = (x @ W) * dinv[node]
  (node-major [100000, 64] f32, 256B rows, row id == original node id), then the
  shards are AllGathered so every core holds the full table in its HBM.
- Each core aggregates its own dst nodes: edges (+self-loops) sorted by
  (512-dst window, src chunk of 25000, dst). dma_gather (int16 local indices,
  256B rows) pulls message rows; per 128-slot tile a one-hot [128,128] is built
  on the DVE (is_equal vs iota) and the PE matmul accumulates into a
  feature-major PSUM window [64 feats, 512 dsts]. Padding slots carry
  dstcol=-1 -> zero one-hot row -> no contribution.
- Window flush: * dinv[dst] (broadcast tile) + bias (+ ReLU for layer 1).
  The flushed hT [64 hid, nodes] directly serves as lhsT for the layer-2 table
  matmul (no transposes anywhere).
- SPMD: one program for all 8 cores; per-(window,chunk) slot counts are the max
  over cores (padded with row-0 gathers / dstcol=-1).
Host side does only sharding/layout/integer structure (edge sort, degree counts,
index arrays); all float math runs on device.
"""
import numpy as np

N = 100000
E = 1600000
FIN = 128
HID = 64
FOUT = 64
NCORES = 8
NSH = N // NCORES           # 12500 nodes per core
NCHUNK = 4
CHS = N // NCHUNK           # 25000 rows per chunk (int16-addressable)
WIN = 512                   # dst nodes per PSUM window
NW = (NSH + WIN - 1) // WIN  # 25 windows
NTILE_NODE = (NSH + 127) // 128  # 98 node tiles per core


def _preprocess(edge_index):
    """Build the common SPMD schedule + per-core index/dstcol arrays."""
    src = np.asarray(edge_index[0], dtype=np.int64)
    dst = np.asarray(edge_index[1], dtype=np.int64)
    # degree includes the self-loop the reference adds; the self term itself is
    # applied locally on-device (no gather slots for loops)
    deg = (np.bincount(dst, minlength=N) + 1).astype(np.float32)

    percore = []
    for c in range(NCORES):
        lo, hi = c * NSH, (c + 1) * NSH
        sel = (dst >= lo) & (dst < hi)
        s, d = src[sel], dst[sel] - lo
        w = d // WIN
        b = s // CHS
        order = np.lexsort((d, b, w))
        s, d, w, b = s[order], d[order], w[order], b[order]
        # counts per (w, b)
        cnt = np.zeros((NW, NCHUNK), np.int64)
        np.add.at(cnt, (w, b), 1)
        percore.append((s, d, cnt))

    cnts = np.stack([pc[2] for pc in percore])      # [8, NW, NCHUNK]
    S_wb = ((cnts.max(axis=0) + 127) // 128) * 128  # common padded slot counts
    starts = np.concatenate([[0], np.cumsum(S_wb.reshape(-1))]).astype(np.int64)
    total_slots = int(starts[-1])
    ntiles_wb = (S_wb // 128).astype(np.int64)

    # fill per-core slot arrays
    gidx = np.zeros((NCORES, total_slots), np.int64)   # local row in chunk (pad=0)
    dcol = np.full((NCORES, total_slots), -1.0, np.float32)  # col - o_j (pad=-1)
    # first pass: raw dst cols to compute common o_j per tile
    rawcol = np.full((NCORES, total_slots), -1, np.int64)
    for c in range(NCORES):
        s, d, cnt = percore[c]
        pos = 0
        for w in range(NW):
            for b in range(NCHUNK):
                n = cnt[w, b]
                base = starts[w * NCHUNK + b]
                gidx[c, base:base + n] = s[pos:pos + n] - b * CHS
                rawcol[c, base:base + n] = d[pos:pos + n] - w * WIN
                pos += n
        assert pos == len(s)

    # dstcol = absolute col within window (pad -1); per-tile list of
    # disjoint (offset, width) one-hot sub-windows covering the tile's span
    dcol[rawcol >= 0] = rawcol[rawcol >= 0].astype(np.float32)
    o_list = []       # per (w,b): list over tiles of [(o, width), ...]
    for w in range(NW):
        for b in range(NCHUNK):
            base = starts[w * NCHUNK + b]
            nt = ntiles_wb[w, b]
            tiles = []
            for j in range(nt):
                seg = rawcol[:, base + j * 128: base + (j + 1) * 128]
                real = seg[seg >= 0]
                if real.size == 0:
                    tiles.append([])  # padding-only tile: no matmul
                    continue
                lo_c, hi_c = int(real.min()), int(real.max())
                assert hi_c - lo_c < 256, f"tile span too wide: {lo_c}..{hi_c}"
                o1 = min(lo_c, WIN - 128)
                sub = [(o1, min(128, WIN - o1))]
                if hi_c >= o1 + 128:
                    o2 = o1 + 128
                    sub.append((o2, min(128, WIN - o2)))
                tiles.append(sub)
            o_list.append(tiles)

    # wrap gidx into dma_gather int16 layout: [128, total/16], 16-part wrap, replicated
    def wrap16(flat):
        n = flat.shape[0]
        wtile = flat.reshape(n // 16, 16).T.astype(np.int16)
        return np.tile(wtile, (8, 1))

    gidx_w = np.stack([wrap16(gidx[c]) for c in range(NCORES)])  # [8,128,total/16]
    # dstcol blocks: slot j*128+p -> [p, j]
    dcol_b = dcol.reshape(NCORES, total_slots // 128, 128).transpose(0, 2, 1).copy()

    return deg, gidx_w, dcol_b, S_wb, starts, ntiles_wb, o_list, total_slots


def _build_program(S_wb, starts, ntiles_wb, o_list, total_slots):
    from concourse import bass, bacc, mybir, tile

    f32 = mybir.dt.float32
    nc = bacc.Bacc(None, target_bir_lowering=False)

    xT = nc.dram_tensor("xT", [FIN, NSH], f32, kind="ExternalInput")
    W1 = nc.dram_tensor("W1", [FIN, HID], f32, kind="ExternalInput")
    W2 = nc.dram_tensor("W2", [HID, FOUT], f32, kind="ExternalInput")
    b1 = nc.dram_tensor("b1", [HID, 1], f32, kind="ExternalInput")
    b2 = nc.dram_tensor("b2", [FOUT, 1], f32, kind="ExternalInput")
    degT = nc.dram_tensor("deg", [128, NTILE_NODE], f32, kind="ExternalInput")
    gidxT = nc.dram_tensor("gidx", [128, total_slots // 16], mybir.dt.int16,
                           kind="ExternalInput")
    dcolT = nc.dram_tensor("dcol", [128, total_slots // 128], f32,
                           kind="ExternalInput")
    outT = nc.dram_tensor("out", [FOUT, NSH], f32, kind="ExternalOutput")

    # internal DRAM
    tab_my = [nc.dram_tensor(f"tab_my{l}", [NSH, HID], f32) for l in (1, 2)]
    tab_full = [nc.dram_tensor(f"tab_full{l}", [N, HID], f32, addr_space="Shared")
                for l in (1, 2)]
    dinv_dram = nc.dram_tensor("dinv_dram", [NTILE_NODE * 128], f32)

    iota_np = np.tile(np.arange(128, dtype=np.float32), (128, 1))
    iota_dram = nc.inline_tensor(iota_np, name="iota128")

    rg = [list(range(NCORES))]

    with tile.TileContext(nc) as tc:
        with (
            tc.tile_pool(name="const", bufs=1) as cpool,
            tc.tile_pool(name="hT", bufs=1) as hpool,
            tc.tile_pool(name="dinvb", bufs=1) as dbpool,
            tc.tile_pool(name="mm", bufs=3) as mmpool,
            tc.tile_pool(name="psA", bufs=2, space="PSUM") as psA,
            tc.tile_pool(name="psB", bufs=2, space="PSUM") as psB,
        ):
            w1t = cpool.tile([FIN, HID], f32)
            nc.sync.dma_start(out=w1t[:], in_=W1[:, :])
            w2t = cpool.tile([HID, FOUT], f32)
            nc.sync.dma_start(out=w2t[:], in_=W2[:, :])
            b1t = cpool.tile([HID, 1], f32)
            nc.sync.dma_start(out=b1t[:], in_=b1[:, :])
            b2t = cpool.tile([FOUT, 1], f32)
            nc.sync.dma_start(out=b2t[:], in_=b2[:, :])
            iot = cpool.tile([128, 128], f32)
            nc.sync.dma_start(out=iot[:], in_=iota_dram[:, :])
            degt = cpool.tile([128, NTILE_NODE], f32)
            nc.sync.dma_start(out=degt[:], in_=degT[:, :])
            dsq = cpool.tile([128, NTILE_NODE], f32)
            nc.scalar.activation(dsq[:], degt[:],
                                 mybir.ActivationFunctionType.Sqrt)
            dinv = cpool.tile([128, NTILE_NODE], f32)
            nc.vector.reciprocal(dinv[:], dsq[:])
            # dinv -> DRAM in node order (node j = tile t, part p -> j = t*128+p)
            nc.sync.dma_start(
                out=dinv_dram.ap().rearrange("(t p) -> p t", p=128), in_=dinv[:])
            # dinv broadcast along partitions [64, NSH]
            dinvb = dbpool.tile([HID, NSH], f32)
            nc.sync.dma_start(out=dinvb[:1, :], in_=dinv_dram.ap()[None, :NSH])
            k = 1
            while k < HID:
                kk = min(k, HID - k)
                nc.sync.dma_start(out=dinvb[k:k + kk, :], in_=dinvb[:kk, :])
                k += kk

            hT = hpool.tile([HID, NTILE_NODE * 128], f32)
            selfT = dbpool.tile([HID, NSH], f32)

            # ---- layer-1 table: tab_my1[n] = (x @ W1)[n] * dinv[n] ----
            with tc.tile_pool(name="xT", bufs=3) as xpool:
                for t in range(NTILE_NODE):
                    n0 = t * 128
                    n1 = min(NSH, n0 + 128)
                    nn = n1 - n0
                    xt = xpool.tile([FIN, 128], f32)
                    nc.sync.dma_start(out=xt[:, :nn], in_=xT[:, n0:n1])
                    ps = psA.tile([128, HID], f32, space="PSUM")
                    nc.tensor.matmul(ps[:nn, :], lhsT=xt[:, :nn], rhs=w1t[:],
                                     start=True, stop=True)
                    sb = mmpool.tile([128, HID], f32)
                    nc.vector.tensor_scalar_mul(sb[:nn, :], ps[:nn, :],
                                                dinv[:nn, t:t + 1])
                    nc.sync.dma_start(out=tab_my[0][n0:n1, :], in_=sb[:nn, :])
                    # self term slice: dinv^2 * (x @ W1).T, feature-major
                    psT = psB.tile([HID, 128], f32, space="PSUM")
                    nc.tensor.matmul(psT[:, :nn], lhsT=w1t[:], rhs=xt[:, :nn],
                                     start=True, stop=True)
                    nc.vector.tensor_mul(selfT[:, n0:n1], psT[:, :nn],
                                         dinvb[:, n0:n1])
                    nc.vector.tensor_mul(selfT[:, n0:n1], selfT[:, n0:n1],
                                         dinvb[:, n0:n1])

            nc.gpsimd.collective_compute(
                "AllGather", mybir.AluOpType.bypass, replica_groups=rg,
                ins=[tab_my[0].ap().opt()], outs=[tab_full[0].ap().opt()])

            # ---- aggregation layers ----
            for layer in (0, 1):
                tabf = tab_full[layer]
                with (
                    tc.tile_pool(name=f"gb{layer}", bufs=3) as gpool,
                    tc.tile_pool(name=f"ix{layer}", bufs=3) as ipool,
                    tc.tile_pool(name=f"dc{layer}", bufs=3) as dpool,
                    tc.tile_pool(name=f"oh{layer}", bufs=4) as ohpool,
                    tc.tile_pool(name=f"fl{layer}", bufs=2) as flpool,
                    tc.tile_pool(name=f"psW{layer}", bufs=2, space="PSUM") as psW,
                ):
                    for w in range(NW):
                        c0 = w * WIN
                        c1 = min(NSH, c0 + WIN)
                        ncol = c1 - c0
                        psw = psW.tile([HID, WIN], f32, space="PSUM")
                        nc.vector.memset(psw[:], 0.0)
                        for b in range(NCHUNK):
                            gi = w * NCHUNK + b
                            S = int(S_wb[w, b])
                            if S == 0:
                                continue
                            base = int(starts[gi])
                            nb = S // 128
                            it = ipool.tile([128, S // 16], mybir.dt.int16)
                            nc.sync.dma_start(
                                out=it[:],
                                in_=gidxT[:, base // 16: base // 16 + S // 16])
                            dt_ = dpool.tile([128, nb], f32)
                            nc.sync.dma_start(
                                out=dt_[:],
                                in_=dcolT[:, base // 128: base // 128 + nb])
                            g = gpool.tile([128, nb * HID], f32)
                            nc.gpsimd.dma_gather(
                                g[:].rearrange("p (n f) -> p n f", n=nb),
                                tabf[b * CHS:(b + 1) * CHS, :],
                                it[:], S, S, HID, single_packet=False)
                            offs = o_list[gi]
                            for j in range(nb):
                                for (o, wd) in offs[j]:
                                    oh = ohpool.tile([128, 128], f32)
                                    nc.vector.scalar_tensor_tensor(
                                        out=oh[:, :wd],
                                        in0=dt_[:, j:j + 1].to_broadcast(
                                            [128, wd]),
                                        scalar=float(o),
                                        in1=iot[:, :wd],
                                        op0=mybir.AluOpType.subtract,
                                        op1=mybir.AluOpType.is_equal)
                                    nc.tensor.matmul(
                                        psw[:, o:o + wd],
                                        lhsT=g[:, j * HID:(j + 1) * HID],
                                        rhs=oh[:, :wd], start=False,
                                        stop=True)
                        # flush window: *dinv[dst] + self-term + bias (+relu)
                        fl = flpool.tile([HID, WIN], f32)
                        nc.vector.tensor_mul(fl[:, :ncol], psw[:, :ncol],
                                             dinvb[:, c0:c1])
                        nc.vector.tensor_add(fl[:, :ncol], fl[:, :ncol],
                                             selfT[:, c0:c1])
                        if layer == 0:
                            nc.scalar.activation(
                                hT[:, c0:c1], fl[:, :ncol],
                                mybir.ActivationFunctionType.Relu,
                                bias=b1t[:])
                        else:
                            nc.vector.tensor_scalar_add(
                                fl[:, :ncol], fl[:, :ncol], b2t[:])
                            nc.sync.dma_start(out=outT[:, c0:c1],
                                              in_=fl[:, :ncol])

                if layer == 0:
                    # layer-2 table: tab_my2[n] = (relu_h @ W2)[n] * dinv[n]
                    for t in range(NTILE_NODE):
                        n0 = t * 128
                        n1 = min(NSH, n0 + 128)
                        nn = n1 - n0
                        ps = psA.tile([128, FOUT], f32, space="PSUM")
                        nc.tensor.matmul(ps[:nn, :], lhsT=hT[:, n0:n1][:, :nn],
                                         rhs=w2t[:], start=True, stop=True)
                        sb = mmpool.tile([128, FOUT], f32)
                        nc.vector.tensor_scalar_mul(sb[:nn, :], ps[:nn, :],
                                                    dinv[:nn, t:t + 1])
                        nc.sync.dma_start(out=tab_my[1][n0:n1, :],
                                          in_=sb[:nn, :])
                        psT = psB.tile([FOUT, 128], f32, space="PSUM")
                        nc.tensor.matmul(psT[:, :nn], lhsT=w2t[:],
                                         rhs=hT[:, n0:n1][:, :nn],
                                         start=True, stop=True)
                        nc.vector.tensor_mul(selfT[:, n0:n1], psT[:, :nn],
                                             dinvb[:, n0:n1])
                        nc.vector.tensor_mul(selfT[:, n0:n1],
                                             selfT[:, n0:n1],
                                             dinvb[:, n0:n1])
                    nc.gpsimd.collective_compute(
                        "AllGather", mybir.AluOpType.bypass, replica_groups=rg,
                        ins=[tab_my[1].ap().opt()],
                        outs=[tab_full[1].ap().opt()])
    nc.compile()
    return nc


TRACE = False        # set True (e.g. from test.py) to capture HW exec time
_LAST_TIMING = None


def kernel(x, edge_index, W1, b1, W2, b2):
    from concourse.bass_utils import run_bass_kernel_spmd

    x = np.asarray(x, np.float32)
    W1 = np.asarray(W1, np.float32)
    W2 = np.asarray(W2, np.float32)
    b1 = np.asarray(b1, np.float32)
    b2 = np.asarray(b2, np.float32)

    deg, gidx_w, dcol_b, S_wb, starts, ntiles_wb, o_list, total_slots = \
        _preprocess(edge_index)

    nc = _build_program(S_wb, starts, ntiles_wb, o_list, total_slots)

    in_maps = []
    for c in range(NCORES):
        lo, hi = c * NSH, (c + 1) * NSH
        degc = deg[lo:hi]
        degp = np.ones(NTILE_NODE * 128, np.float32)
        degp[:NSH] = degc
        in_maps.append({
            "xT": np.ascontiguousarray(x[lo:hi].T),
            "W1": W1, "W2": W2,
            "b1": b1.reshape(HID, 1), "b2": b2.reshape(FOUT, 1),
            "deg": np.ascontiguousarray(degp.reshape(NTILE_NODE, 128).T),
            "gidx": gidx_w[c],
            "dcol": dcol_b[c],
        })

    kwargs = {"trace": True} if TRACE else {}
    res = run_bass_kernel_spmd(nc, in_maps, core_ids=list(range(NCORES)),
                               **kwargs)
    globals()["_LAST_TIMING"] = getattr(res, "exec_time_ns", None)

    z = np.empty((N, FOUT), np.float32)
    for c in range(NCORES):
        lo, hi = c * NSH, (c + 1) * NSH
        z[lo:hi] = np.asarray(res.results[c]["out"]).reshape(FOUT, NSH).T
    return z



# revision 4
# speedup vs baseline: 1.5348x; 1.5348x over previous
"""2-layer GCN on 8 Trainium2 NeuronCores.

Strategy (edge-cut node sharding):
- 8 cores, core c owns dst nodes [c*12500, (c+1)*12500).
- Per layer, each core builds its shard of the message table xs = (x @ W) * dinv[node]
  (node-major [100000, 64] f32, 256B rows), shards AllGathered so every core
  holds the full table in HBM.
- Each core aggregates its own dst nodes: edges (+self-loops) sorted by
  (window-PAIR of 1024 dsts, src chunk of 25000, window, dst). dma_gather
  (int16 local indices, 256B rows) pulls message rows with one call per
  (window-pair, chunk) [S ~= 4300] alternating between TWO SWDGE queues so
  descriptor generation overlaps on the GpSimd engine (the bottleneck:
  ~5ns/row vs ~8.5ns single-queue). Per 128-slot tile a one-hot [128,128]
  is built on the DVE (is_equal vs iota) and the PE matmul accumulates into
  the owning window's feature-major PSUM tile [64 feats, 512 dsts].
  Padding slots carry dstcol=-1 -> zero one-hot row -> no contribution.
- Window flush: * dinv[dst] (broadcast tile) + self-term + bias (+ ReLU layer 1).
  Flushed hT [64, nodes] directly serves as lhsT for the layer-2 table matmul.
- Table builds load x / store tables with few big DMAs (bulk staging) to keep
  the serial table phases short.
- SPMD: one program for all 8 cores; per-(window,chunk) slot counts are the max
  over cores (padded with row-0 gathers / dstcol=-1).
Host side does only sharding/layout/integer structure (edge sort, degree counts,
index arrays); all float math runs on device.
"""
import numpy as np

N = 100000
E = 1600000
FIN = 128
HID = 64
FOUT = 64
NCORES = 8
NSH = N // NCORES           # 12500 nodes per core
NCHUNK = 4
CHS = N // NCHUNK           # 25000 rows per chunk (int16-addressable)
WIN = 512                   # dst nodes per PSUM window
NW = (NSH + WIN - 1) // WIN  # 25 windows
NWP = (NW + 1) // 2          # 13 window pairs (last is a single window)
NTILE_NODE = (NSH + 127) // 128  # 98 node tiles per core


def _preprocess(edge_index):
    """Build the common SPMD schedule + per-core index/dstcol arrays.

    Slot order: (window-pair wp, chunk b, window w, dst). Each (w, b) segment
    is padded to a multiple of 128 (max count over cores), so every 128-slot
    tile belongs to exactly one window.
    """
    src = np.asarray(edge_index[0], dtype=np.int64)
    dst = np.asarray(edge_index[1], dtype=np.int64)
    deg = (np.bincount(dst, minlength=N) + 1).astype(np.float32)

    percore = []
    for c in range(NCORES):
        lo, hi = c * NSH, (c + 1) * NSH
        sel = (dst >= lo) & (dst < hi)
        s, d = src[sel], dst[sel] - lo
        w = d // WIN
        b = s // CHS
        wp = w // 2
        order = np.lexsort((d, w, b, wp))
        s, d, w, b = s[order], d[order], w[order], b[order]
        cnt = np.zeros((NW, NCHUNK), np.int64)
        np.add.at(cnt, (w, b), 1)
        percore.append((s, d, cnt))

    cnts = np.stack([pc[2] for pc in percore])      # [8, NW, NCHUNK]
    S_wb = ((cnts.max(axis=0) + 127) // 128) * 128  # padded per (w, b)

    # segment start offsets in (wp, b, w) order
    seg_start = np.zeros((NW, NCHUNK), np.int64)
    pos = 0
    for wp in range(NWP):
        ws = [2 * wp] + ([2 * wp + 1] if 2 * wp + 1 < NW else [])
        for b in range(NCHUNK):
            for w in ws:
                seg_start[w, b] = pos
                pos += S_wb[w, b]
    total_slots = int(pos)

    gidx = np.zeros((NCORES, total_slots), np.int64)   # local row in chunk
    dcol = np.full((NCORES, total_slots), -1.0, np.float32)
    rawcol = np.full((NCORES, total_slots), -1, np.int64)
    for c in range(NCORES):
        s, d, cnt = percore[c]
        # slots for this core appear in (wp, b, w, d) order already
        pos_c = 0
        for wp in range(NWP):
            ws = [2 * wp] + ([2 * wp + 1] if 2 * wp + 1 < NW else [])
            for b in range(NCHUNK):
                for w in ws:
                    n = cnt[w, b]
                    base = seg_start[w, b]
                    gidx[c, base:base + n] = s[pos_c:pos_c + n] - b * CHS
                    rawcol[c, base:base + n] = d[pos_c:pos_c + n] - w * WIN
                    pos_c += n
        assert pos_c == len(s)

    dcol[rawcol >= 0] = rawcol[rawcol >= 0].astype(np.float32)

    # per merged call (wp, b): tiles with (window-parity dw, [(o, wd), ...])
    call_S = np.zeros((NWP, NCHUNK), np.int64)
    call_start = np.zeros((NWP, NCHUNK), np.int64)
    o_list = []   # [NWP * NCHUNK] -> list over tiles of (dw, [(o, wd), ...])
    for wp in range(NWP):
        ws = [2 * wp] + ([2 * wp + 1] if 2 * wp + 1 < NW else [])
        for b in range(NCHUNK):
            call_start[wp, b] = seg_start[ws[0], b]
            call_S[wp, b] = sum(int(S_wb[w, b]) for w in ws)
            tiles = []
            for wi, w in enumerate(ws):
                base = seg_start[w, b]
                nt = int(S_wb[w, b]) // 128
                for j in range(nt):
                    seg = rawcol[:, base + j * 128: base + (j + 1) * 128]
                    real = seg[seg >= 0]
                    if real.size == 0:
                        tiles.append((wi, []))
                        continue
                    lo_c, hi_c = int(real.min()), int(real.max())
                    assert hi_c - lo_c < 256
                    o1 = min(lo_c, WIN - 128)
                    sub = [(o1, min(128, WIN - o1))]
                    if hi_c >= o1 + 128:
                        o2 = o1 + 128
                        sub.append((o2, min(128, WIN - o2)))
                    tiles.append((wi, sub))
            o_list.append(tiles)

    def wrap16(flat):
        n = flat.shape[0]
        wtile = flat.reshape(n // 16, 16).T.astype(np.int16)
        return np.tile(wtile, (8, 1))

    gidx_w = np.stack([wrap16(gidx[c]) for c in range(NCORES)])
    dcol_b = dcol.reshape(NCORES, total_slots // 128, 128).transpose(0, 2, 1).copy()

    return deg, gidx_w, dcol_b, call_S, call_start, o_list, total_slots


def _build_program(call_S, call_start, o_list, total_slots):
    from concourse import bass, bacc, mybir, tile

    f32 = mybir.dt.float32
    nc = bacc.Bacc(None, target_bir_lowering=False, num_swdge_queues=2)

    xT = nc.dram_tensor("xT", [FIN, NSH], f32, kind="ExternalInput")
    W1 = nc.dram_tensor("W1", [FIN, HID], f32, kind="ExternalInput")
    W2 = nc.dram_tensor("W2", [HID, FOUT], f32, kind="ExternalInput")
    b1 = nc.dram_tensor("b1", [HID, 1], f32, kind="ExternalInput")
    b2 = nc.dram_tensor("b2", [FOUT, 1], f32, kind="ExternalInput")
    degT = nc.dram_tensor("deg", [128, NTILE_NODE], f32, kind="ExternalInput")
    gidxT = nc.dram_tensor("gidx", [128, total_slots // 16], mybir.dt.int16,
                           kind="ExternalInput")
    dcolT = nc.dram_tensor("dcol", [128, total_slots // 128], f32,
                           kind="ExternalInput")
    outT = nc.dram_tensor("out", [FOUT, NSH], f32, kind="ExternalOutput")

    tab_my = [nc.dram_tensor(f"tab_my{l}", [NSH, HID], f32) for l in (1, 2)]
    tab_full = [nc.dram_tensor(f"tab_full{l}", [N, HID], f32, addr_space="Shared")
                for l in (1, 2)]
    dinv_dram = nc.dram_tensor("dinv_dram", [NTILE_NODE * 128], f32)

    iota_np = np.tile(np.arange(128, dtype=np.float32), (128, 1))
    iota_dram = nc.inline_tensor(iota_np, name="iota128")

    rg = [list(range(NCORES))]
    NFULL = 97            # node tiles stored via one bulk DMA (97*128 = 12416)
    NREM = NSH - NFULL * 128   # 84 rows in the last (partial) tile

    with tile.TileContext(nc) as tc:
        with (
            tc.tile_pool(name="const", bufs=1) as cpool,
            tc.tile_pool(name="hT", bufs=1) as hpool,
            tc.tile_pool(name="dinvb", bufs=1) as dbpool,
            tc.tile_pool(name="mm", bufs=3) as mmpool,
            tc.tile_pool(name="psA", bufs=2, space="PSUM") as psA,
            tc.tile_pool(name="psB", bufs=2, space="PSUM") as psB,
        ):
            w1t = cpool.tile([FIN, HID], f32)
            nc.sync.dma_start(out=w1t[:], in_=W1[:, :])
            w2t = cpool.tile([HID, FOUT], f32)
            nc.sync.dma_start(out=w2t[:], in_=W2[:, :])
            b1t = cpool.tile([HID, 1], f32)
            nc.sync.dma_start(out=b1t[:], in_=b1[:, :])
            b2t = cpool.tile([FOUT, 1], f32)
            nc.sync.dma_start(out=b2t[:], in_=b2[:, :])
            iot = cpool.tile([128, 128], f32)
            nc.sync.dma_start(out=iot[:], in_=iota_dram[:, :])
            degt = cpool.tile([128, NTILE_NODE], f32)
            nc.sync.dma_start(out=degt[:], in_=degT[:, :])
            dsq = cpool.tile([128, NTILE_NODE], f32)
            nc.scalar.activation(dsq[:], degt[:],
                                 mybir.ActivationFunctionType.Sqrt)
            dinv = cpool.tile([128, NTILE_NODE], f32)
            nc.vector.reciprocal(dinv[:], dsq[:])
            nc.sync.dma_start(
                out=dinv_dram.ap().rearrange("(t p) -> p t", p=128), in_=dinv[:])
            dinvb = dbpool.tile([HID, NSH], f32)
            nc.sync.dma_start(out=dinvb[:1, :], in_=dinv_dram.ap()[None, :NSH])
            k = 1
            while k < HID:
                kk = min(k, HID - k)
                nc.sync.dma_start(out=dinvb[k:k + kk, :], in_=dinvb[:kk, :])
                k += kk

            hT = hpool.tile([HID, NTILE_NODE * 128], f32)
            selfT = dbpool.tile([HID, NSH], f32)

            # ---- layer-1 table: tab_my1[n] = (x @ W1)[n] * dinv[n] ----
            XB = 8  # node tiles per bulk x load
            with (
                tc.tile_pool(name="xT", bufs=2) as xpool,
                tc.tile_pool(name="st", bufs=1) as spool,
            ):
                sbTab = spool.tile([128, NTILE_NODE * HID], f32)
                for t0 in range(0, NTILE_NODE, XB):
                    t1 = min(NTILE_NODE, t0 + XB)
                    n0, n1 = t0 * 128, min(NSH, t1 * 128)
                    xt = xpool.tile([FIN, XB * 128], f32)
                    nc.sync.dma_start(out=xt[:, :n1 - n0], in_=xT[:, n0:n1])
                    for t in range(t0, t1):
                        m0 = t * 128
                        nn = min(NSH, m0 + 128) - m0
                        sl = xt[:, (m0 - n0):(m0 - n0) + nn]
                        ps = psA.tile([128, HID], f32, space="PSUM")
                        nc.tensor.matmul(ps[:nn, :], lhsT=sl, rhs=w1t[:],
                                         start=True, stop=True)
                        nc.vector.tensor_scalar_mul(
                            sbTab[:nn, t * HID:(t + 1) * HID], ps[:nn, :],
                            dinv[:nn, t:t + 1])
                        psT = psB.tile([HID, 128], f32, space="PSUM")
                        nc.tensor.matmul(psT[:, :nn], lhsT=w1t[:], rhs=sl,
                                         start=True, stop=True)
                        nc.vector.tensor_mul(selfT[:, m0:m0 + nn], psT[:, :nn],
                                             dinvb[:, m0:m0 + nn])
                        nc.vector.tensor_mul(selfT[:, m0:m0 + nn],
                                             selfT[:, m0:m0 + nn],
                                             dinvb[:, m0:m0 + nn])
                nc.sync.dma_start(
                    out=tab_my[0].ap()[:NFULL * 128, :].rearrange(
                        "(t p) f -> p t f", p=128),
                    in_=sbTab[:, :NFULL * HID].rearrange(
                        "p (t f) -> p t f", t=NFULL))
                nc.sync.dma_start(
                    out=tab_my[0].ap()[NFULL * 128:, :],
                    in_=sbTab[:NREM, NFULL * HID:NTILE_NODE * HID])

            nc.gpsimd.collective_compute(
                "AllGather", mybir.AluOpType.bypass, replica_groups=rg,
                ins=[tab_my[0].ap().opt()], outs=[tab_full[0].ap().opt()])

            # ---- aggregation layers ----
            for layer in (0, 1):
                tabf = tab_full[layer]
                call_idx = 0
                with (
                    tc.tile_pool(name=f"gb{layer}", bufs=3) as gpool,
                    tc.tile_pool(name=f"ix{layer}", bufs=3) as ipool,
                    tc.tile_pool(name=f"dc{layer}", bufs=3) as dpool,
                    tc.tile_pool(name=f"oh{layer}", bufs=4) as ohpool,
                    tc.tile_pool(name=f"fl{layer}", bufs=2) as flpool,
                    tc.tile_pool(name=f"psW{layer}", bufs=4, space="PSUM") as psW,
                ):
                    for wp in range(NWP):
                        ws = [2 * wp] + ([2 * wp + 1] if 2 * wp + 1 < NW else [])
                        psws = []
                        for _ in ws:
                            p = psW.tile([HID, WIN], f32, space="PSUM")
                            nc.vector.memset(p[:], 0.0)
                            psws.append(p)
                        for b in range(NCHUNK):
                            gi = wp * NCHUNK + b
                            S = int(call_S[wp, b])
                            if S == 0:
                                continue
                            base = int(call_start[wp, b])
                            nb = S // 128
                            it = ipool.tile([128, S // 16], mybir.dt.int16)
                            nc.sync.dma_start(
                                out=it[:],
                                in_=gidxT[:, base // 16: base // 16 + S // 16])
                            dt_ = dpool.tile([128, nb], f32)
                            nc.sync.dma_start(
                                out=dt_[:],
                                in_=dcolT[:, base // 128: base // 128 + nb])
                            g = gpool.tile([128, nb * HID], f32)
                            nc.gpsimd.dma_gather(
                                g[:].rearrange("p (n f) -> p n f", n=nb),
                                tabf[b * CHS:(b + 1) * CHS, :],
                                it[:], S, S, HID, single_packet=False,
                                queue_num=call_idx % 2)
                            call_idx += 1
                            tiles = o_list[gi]
                            for j in range(nb):
                                dw, offs = tiles[j]
                                for (o, wd) in offs:
                                    oh = ohpool.tile([128, 128], f32)
                                    nc.vector.scalar_tensor_tensor(
                                        out=oh[:, :wd],
                                        in0=dt_[:, j:j + 1].to_broadcast(
                                            [128, wd]),
                                        scalar=float(o),
                                        in1=iot[:, :wd],
                                        op0=mybir.AluOpType.subtract,
                                        op1=mybir.AluOpType.is_equal)
                                    nc.tensor.matmul(
                                        psws[dw][:, o:o + wd],
                                        lhsT=g[:, j * HID:(j + 1) * HID],
                                        rhs=oh[:, :wd], start=False,
                                        stop=True)
                        # flush both windows of the pair
                        for wi, w in enumerate(ws):
                            c0 = w * WIN
                            c1 = min(NSH, c0 + WIN)
                            ncol = c1 - c0
                            fl = flpool.tile([HID, WIN], f32)
                            nc.vector.tensor_mul(fl[:, :ncol],
                                                 psws[wi][:, :ncol],
                                                 dinvb[:, c0:c1])
                            nc.vector.tensor_add(fl[:, :ncol], fl[:, :ncol],
                                                 selfT[:, c0:c1])
                            if layer == 0:
                                nc.scalar.activation(
                                    hT[:, c0:c1], fl[:, :ncol],
                                    mybir.ActivationFunctionType.Relu,
                                    bias=b1t[:])
                            else:
                                nc.vector.tensor_scalar_add(
                                    fl[:, :ncol], fl[:, :ncol], b2t[:])
                                nc.sync.dma_start(out=outT[:, c0:c1],
                                                  in_=fl[:, :ncol])

                if layer == 0:
                    # layer-2 table: tab_my2[n] = (relu_h @ W2)[n] * dinv[n]
                    with tc.tile_pool(name="st2", bufs=1) as spool2:
                        sbTab2 = spool2.tile([128, NTILE_NODE * HID], f32)
                        for t in range(NTILE_NODE):
                            n0 = t * 128
                            nn = min(NSH, n0 + 128) - n0
                            ps = psA.tile([128, FOUT], f32, space="PSUM")
                            nc.tensor.matmul(ps[:nn, :],
                                             lhsT=hT[:, n0:n0 + nn],
                                             rhs=w2t[:], start=True, stop=True)
                            nc.vector.tensor_scalar_mul(
                                sbTab2[:nn, t * FOUT:(t + 1) * FOUT],
                                ps[:nn, :], dinv[:nn, t:t + 1])
                            psT = psB.tile([FOUT, 128], f32, space="PSUM")
                            nc.tensor.matmul(psT[:, :nn], lhsT=w2t[:],
                                             rhs=hT[:, n0:n0 + nn],
                                             start=True, stop=True)
                            nc.vector.tensor_mul(selfT[:, n0:n0 + nn],
                                                 psT[:, :nn],
                                                 dinvb[:, n0:n0 + nn])
                            nc.vector.tensor_mul(selfT[:, n0:n0 + nn],
                                                 selfT[:, n0:n0 + nn],
                                                 dinvb[:, n0:n0 + nn])
                        nc.sync.dma_start(
                            out=tab_my[1].ap()[:NFULL * 128, :].rearrange(
                                "(t p) f -> p t f", p=128),
                            in_=sbTab2[:, :NFULL * HID].rearrange(
                                "p (t f) -> p t f", t=NFULL))
                        nc.sync.dma_start(
                            out=tab_my[1].ap()[NFULL * 128:, :],
                            in_=sbTab2[:NREM, NFULL * HID:NTILE_NODE * HID])
                    nc.gpsimd.collective_compute(
                        "AllGather", mybir.AluOpType.bypass, replica_groups=rg,
                        ins=[tab_my[1].ap().opt()],
                        outs=[tab_full[1].ap().opt()])
    nc.compile()
    return nc


TRACE = False        # set True (e.g. from test.py) to capture HW exec time
_LAST_TIMING = None


def kernel(x, edge_index, W1, b1, W2, b2):
    from concourse.bass_utils import run_bass_kernel_spmd

    x = np.asarray(x, np.float32)
    W1 = np.asarray(W1, np.float32)
    W2 = np.asarray(W2, np.float32)
    b1 = np.asarray(b1, np.float32)
    b2 = np.asarray(b2, np.float32)

    deg, gidx_w, dcol_b, call_S, call_start, o_list, total_slots = \
        _preprocess(edge_index)

    nc = _build_program(call_S, call_start, o_list, total_slots)

    in_maps = []
    for c in range(NCORES):
        lo, hi = c * NSH, (c + 1) * NSH
        degc = deg[lo:hi]
        degp = np.ones(NTILE_NODE * 128, np.float32)
        degp[:NSH] = degc
        in_maps.append({
            "xT": np.ascontiguousarray(x[lo:hi].T),
            "W1": W1, "W2": W2,
            "b1": b1.reshape(HID, 1), "b2": b2.reshape(FOUT, 1),
            "deg": np.ascontiguousarray(degp.reshape(NTILE_NODE, 128).T),
            "gidx": gidx_w[c],
            "dcol": dcol_b[c],
        })

    kwargs = {"trace": True} if TRACE else {}
    res = run_bass_kernel_spmd(nc, in_maps, core_ids=list(range(NCORES)),
                               **kwargs)
    globals()["_LAST_TIMING"] = getattr(res, "exec_time_ns", None)

    z = np.empty((N, FOUT), np.float32)
    for c in range(NCORES):
        lo, hi = c * NSH, (c + 1) * NSH
        z[lo:hi] = np.asarray(res.results[c]["out"]).reshape(FOUT, NSH).T
    return z
